# revision 36
# baseline (speedup 1.0000x reference)
"""Trainium2 Bass kernel for nn_Attention (B=2, S=2048, D=1024, H=16, causal).

Sharding: head-parallel across 8 NeuronCores - 2 heads per core. Each core:
  1. computes qT/kT/vT for its 2 heads from the full xT (QKV projection,
     transposed layout [128 = 2*hd, S]), in bf16,
  2. runs causal attention per head with scores in transposed orientation
     (sT[sj, si]) so the PV matmul needs no P transpose; the softmax
     denominator comes free as an extra ones-column in the V operand,
  3. multiplies by its 128-row slice of W_proj producing a partial output
     yT_c [B, D, S] in bf16.
Host sums the 8 partials, adds b_proj, and transposes back to [B, S, D].

Scheduling: QKV(b1) matmul groups are interleaved into attention(b0) and
proj(b0) into attention(b1) so the PE never idles while the Activation
engine churns through the softmax exps.
"""
import sys

sys.path.insert(0, "/opt/trn_rl_repo")

import numpy as np
import concourse.bacc as bacc
import concourse.mybir as mybir
import concourse.tile as tile
from concourse.bass_utils import run_bass_kernel_spmd

dt = mybir.dt
BF16 = dt.bfloat16
F32R = dt.float32r
AF = mybir.ActivationFunctionType

B, S, D, H = 2, 2048, 1024, 16
HD = D // H            # 64
NCORE = 8
HPC = H // NCORE       # 2 heads per core

_CACHE = {}


def build_nc():
    nc = bacc.Bacc("TRN2", target_bir_lowering=False, debug=False)

    xT_d = nc.dram_tensor("xT", [B, D, S], BF16, kind="ExternalInput")
    wq_d = nc.dram_tensor("wq", [128, 8, 128], BF16, kind="ExternalInput")
    wk_d = nc.dram_tensor("wk", [128, 8, 128], BF16, kind="ExternalInput")
    wv_d = nc.dram_tensor("wv", [128, 8, 128], BF16, kind="ExternalInput")
    bq_d = nc.dram_tensor("bq", [128, 1], dt.float32, kind="ExternalInput")
    bk_d = nc.dram_tensor("bk", [128, 1], dt.float32, kind="ExternalInput")
    bv_d = nc.dram_tensor("bv", [128, 1], dt.float32, kind="ExternalInput")
    wp_d = nc.dram_tensor("wp", [128, D], BF16, kind="ExternalInput")
    negm_d = nc.dram_tensor("negm2", [128, 128], BF16, kind="ExternalInput")
    id_d = nc.dram_tensor("ident", [128, 128], BF16, kind="ExternalInput")
    ones_d = nc.dram_tensor("ones", [128, 32], BF16, kind="ExternalInput")
    l2r_d = nc.dram_tensor("l2r", [2, 128], F32R, kind="ExternalInput")
    yT_d = nc.dram_tensor("yT", [B, D, S], BF16, kind="ExternalOutput")

    with tile.TileContext(nc) as tc:
        with (
            tc.tile_pool(name="consts", bufs=1) as consts,
            tc.tile_pool(name="xpool", bufs=2) as xpool,
            tc.tile_pool(name="qkv", bufs=2) as qkvp,
            tc.tile_pool(name="epool", bufs=3) as epool,
            tc.tile_pool(name="ypool", bufs=6) as ypool,
            tc.tile_pool(name="rpool", bufs=3) as rpool,
            tc.tile_pool(name="ps_mm2", bufs=3, space="PSUM") as ps_mm2,
            tc.tile_pool(name="ps_acc", bufs=2, space="PSUM") as ps_acc,
        ):
            # ---- constants / weights: issued on the scalar (Act) queue,
            # which is otherwise idle until the first exps ~15us in. The
            # v-path weights go first (first QKV quarter is v).
            wqr = consts.tile([128, 8, 128], BF16, tag="wq")
            wkr = consts.tile([128, 8, 128], BF16, tag="wk")
            wvr = consts.tile([128, 8, 128], BF16, tag="wv")
            wpr = consts.tile([128, D], BF16, tag="wp")
            bq_sb = consts.tile([128, 1], dt.float32, tag="bq")
            bk_sb = consts.tile([128, 1], dt.float32, tag="bk")
            bv_sb = consts.tile([128, 1], dt.float32, tag="bv")
            negm2 = consts.tile([128, 128], BF16, tag="negm2")
            ident = consts.tile([128, 128], BF16, tag="ident")
            l2r = consts.tile([2, 128], F32R, tag="l2r")
            nc.scalar.dma_start(wvr[:], wv_d.ap()[:])
            nc.scalar.dma_start(bv_sb[:], bv_d.ap()[:])
            nc.sync.dma_start(ident[:], id_d.ap()[:])
            nc.sync.dma_start(negm2[:], negm_d.ap()[:])

            # preload the ln+exp activation table once: the auto-placement
            # pass picks per-func first-match tables and would thrash
            # Exp <-> Ln every block otherwise.
            from concourse.hw_specs import get_activation_tables
            _tabs = list(get_activation_tables(nc.m.arch).items())
            _combined = next(i for i, (_n, _s) in enumerate(_tabs)
                             if AF.Exp in _s and AF.Ln in _s)
            nc.scalar.add_instruction(mybir.InstLoadActFuncSet(
                name=nc.get_next_instruction_name(),
                act_func_set_id=_combined, ins=[], outs=[]))

            # ---- x tiles: [128, S] per (b, dtile); two half-loads each.
            # b0's first halves ride the fast-starting vector/scalar queues
            # (the sync queue only comes up ~7us in) so the first QKV
            # matmuls can begin ~2us after launch.
            xd = {}
            for b in range(B):
                for d8 in range(8):
                    t = xpool.tile([128, S], BF16, tag=f"x{d8}",
                                   name=f"x_{b}_{d8}")
                    for h in range(2):
                        if b == 0 and h == 0:
                            eng = nc.gpsimd if d8 < 4 else nc.scalar
                        else:
                            eng = nc.sync if (d8 + h) % 2 == 0 else nc.gpsimd
                        eng.dma_start(
                            t[:, 1024 * h:1024 * (h + 1)],
                            xT_d.ap()[b, 128 * d8:128 * (d8 + 1),
                                      1024 * h:1024 * (h + 1)])
                    xd[(b, d8)] = t

            # remaining constants, after the critical x halves. Only what
            # the first scores need stays on scalar (the Act queue must
            # reach exp(0) fast); the rest rides sync, which is idle after
            # the x loads.
            nc.scalar.dma_start(wkr[:], wk_d.ap()[:])
            nc.scalar.dma_start(bk_sb[:], bk_d.ap()[:])
            nc.scalar.dma_start(wqr[:], wq_d.ap()[:])
            nc.scalar.dma_start(bq_sb[:], bq_d.ap()[:])
            nc.sync.dma_start(wpr[:], wp_d.ap()[:])
            nc.sync.dma_start(l2r[:], l2r_d.ap()[:])

            qT, kT, vT, vhat, aT = {}, {}, {}, {}, {}
            for b in range(B):
                qT[b] = qkvp.tile([128, S], BF16, tag="qT", name=f"qT_{b}")
                kT[b] = qkvp.tile([128, S], BF16, tag="kT", name=f"kT_{b}")
                vT[b] = qkvp.tile([128, S], BF16, tag="vT", name=f"vT_{b}")
                vhat[b] = qkvp.tile([128, 16, 130], BF16, tag="vhat",
                                    name=f"vhat_{b}")
                aT[b] = qkvp.tile([128, S], BF16, tag="aT", name=f"aT_{b}")
                # ones columns of vhat (softmax denominator rows).
                # NB: this strided scatter must stay off the sync HWDGE
                # queue (produced zero columns there -> inf); scalar works.
                nc.scalar.dma_start(vhat[b][:, :, 64], ones_d.ap()[:, 0:16])
                nc.scalar.dma_start(vhat[b][:, :, 129], ones_d.ap()[:, 16:32])

            KINDS = {"q": (wqr, bq_sb, qT), "k": (wkr, bk_sb, kT),
                     "v": (wvr, bv_sb, vT)}

            def emit_qkv_quarter(b, kind, qi):
                """One [128, 512] quarter of q/k/v for batch b; 8 matmuls +
                one DVE bias-add drain."""
                w_r, bias, dst = KINDS[kind]
                pp = ps_mm2.tile([128, 2, 512], dt.float32, tag="mm2",
                                 name=f"qkv_{b}_{kind}_{qi}")
                cols = slice(512 * qi, 512 * (qi + 1))
                for d8 in range(8):
                    nc.tensor.matmul(
                        pp[:, 0, :],
                        w_r[:, d8, :],
                        xd[(b, d8)][:, cols],
                        start=(d8 == 0),
                        stop=(d8 == 7),
                    )
                with nc.allow_low_precision(reason="bf16 qkv bias"):
                    nc.vector.tensor_scalar_add(
                        dst[b][:, cols], pp[:, 0, :], bias[:, 0:1])

            def emit_vhat(b, j):
                """v natural-orientation tile j + ones column via PE
                transpose."""
                pst = ps_mm2.tile([128, 128], BF16, tag="mm2",
                                  name=f"tr_{b}_{j}")
                nc.tensor.transpose(
                    pst[:], vT[b][:, 128 * j:128 * (j + 1)], ident[:]
                )
                with nc.allow_low_precision(reason="bf16 vhat"):
                    nc.vector.tensor_copy(vhat[b][:, j, 0:64], pst[:, 0:64])
                    nc.vector.tensor_copy(vhat[b][:, j, 65:129],
                                          pst[:, 64:128])

            def emit_scores(b, blk, j):
                """Scores sT[sj, si] for both heads of j-tile vs query block
                blk, exp'd into a bf16 ee tile (diag tiles causally
                masked)."""
                si0 = 512 * blk
                off = max(0, 128 * (j - 4 * blk))
                w = 512 - off
                pp = ps_mm2.tile([128, 2, 512], dt.float32, tag="mm2",
                                 name=f"pp_{b}_{blk}_{j}")
                for hl in range(HPC):
                    rows = slice(64 * hl, 64 * (hl + 1))
                    nc.tensor.matmul(
                        pp[:, hl, 0:w],
                        kT[b][rows, 128 * j:128 * (j + 1)],
                        qT[b][rows, si0 + off:si0 + 512],
                        start=True,
                        stop=True,
                    )
                ee = epool.tile([128, 2, 512], BF16, tag="eT",
                                name=f"ee_{b}_{blk}_{j}")
                nc.scalar.activation(
                    ee[:, :, 0:w], pp[:, :, 0:w], AF.Exp, scale=0.125
                )
                if j >= 4 * blk:
                    with nc.allow_low_precision(reason="causal mask"):
                        for hl in range(HPC):
                            nc.vector.tensor_mul(
                                ee[:, hl, 0:128], ee[:, hl, 0:128], negm2[:]
                            )
                return ee

            def emit_pv(b, blk, j, psa, ee):
                off = max(0, 128 * (j - 4 * blk))
                w = 512 - off
                jlast = 4 * blk + 3
                for hl in range(HPC):
                    nc.tensor.matmul(
                        psa[hl][:, off:512],
                        vhat[b][:, j, 65 * hl:65 * hl + 65],
                        ee[:, hl, 0:w],
                        start=(j == 0),
                        stop=(j == jlast),
                    )

            def emit_norm(b, blk, psa):
                """aT[:, blk] = psa rows / softmax denominator (psa row 64),
                via DVE reciprocal + PE broadcast."""
                si0 = 512 * blk
                # 1/d via exp(-ln d): Ln rows, col-tiled PE broadcast (the
                # two matmuls run concurrently), one Exp drain -- all on the
                # single preloaded ln+exp act table.
                lnl = [
                    rpool.tile([1, 512], F32R, tag=f"lnl{hl}",
                               name=f"lnl_{b}_{blk}_{hl}")
                    for hl in range(HPC)
                ]
                for hl in range(HPC):
                    nc.scalar.activation(lnl[hl][:], psa[hl][64:65, :], AF.Ln)
                rec_sb = []
                for hl in range(HPC):
                    psb = ps_mm2.tile([64, 512], dt.float32, tag="mm2",
                                      name=f"psb_{b}_{blk}_{hl}")
                    nc.tensor.matmul(psb[:], l2r[0:1, 0:64], lnl[hl][:],
                                     start=True, stop=True)
                    rs = rpool.tile([64, 512], BF16, tag=f"recs{hl}",
                                    name=f"recs_{b}_{blk}_{hl}")
                    nc.scalar.activation(rs[:], psb[:], AF.Exp, scale=-1.0)
                    rec_sb.append(rs)
                with nc.allow_low_precision(reason="bf16 attn normalize"):
                    for hl in range(HPC):
                        p0 = 64 * hl
                        nc.vector.tensor_mul(
                            aT[b][p0:p0 + 64, si0:si0 + 512],
                            psa[hl][0:64, :],
                            rec_sb[hl][:],
                        )

            _proj_n = [0]

            def emit_proj_tile(b, blk, dtile):
                si0 = 512 * blk
                ps = ps_mm2.tile([128, 512], dt.float32, tag="mm2",
                                 name=f"psp_{b}_{blk}_{dtile}")
                nc.tensor.matmul(
                    ps[:],
                    wpr[:, 128 * dtile:128 * (dtile + 1)],
                    aT[b][:, si0:si0 + 512],
                    start=True,
                    stop=True,
                )
                y_sb = ypool.tile([128, 512], BF16, tag="y",
                                  name=f"y_{b}_{blk}_{dtile}")
                n = _proj_n[0]
                _proj_n[0] += 1
                # 1-in-3 drains on Act to balance DVE (which also carries
                # masks/muls); GPSIMD has no PSUM port.
                with nc.allow_low_precision(reason="bf16 y drain"):
                    if n % 3 == 0 or (b == 1 and blk == 0 and n % 2 == 0):
                        nc.scalar.activation(y_sb[:], ps[:], AF.Copy)
                    else:
                        nc.vector.tensor_copy(y_sb[:], ps[:])
                dma_eng = nc.sync if n % 2 == 0 else nc.gpsimd
                dma_eng.dma_start(
                    yT_d.ap()[b, 128 * dtile:128 * (dtile + 1),
                              si0:si0 + 512],
                    y_sb[:],
                )

            _vhat_done = {0: set(), 1: set()}

            def emit_attn_block(b, blk, fillers, pre=()):
                """One 512-query attention block; `fillers` is a list of
                zero-arg emit callbacks sprinkled into the PE stream to keep
                the PE busy while Act runs the exps. `pre` callbacks run
                right after the first scores, before the vhat transposes
                (e.g. the v quarter those transposes consume)."""
                jlast = 4 * blk + 3
                psa = [
                    ps_acc.tile([65, 512], dt.float32, tag="acc",
                                name=f"psa_{b}_{blk}_{hl}")
                    for hl in range(HPC)
                ]
                fi = 0
                nj = jlast + 1
                ees = {}
                for j in range(nj):
                    ees[j] = emit_scores(b, blk, j)
                    if j == 0:
                        for p in pre:
                            p()
                        # vhat transposes after the first scores so Act's
                        # first exp of the block isn't delayed by them
                        for jj in range(jlast + 1):
                            if jj not in _vhat_done[b]:
                                emit_vhat(b, jj)
                                _vhat_done[b].add(jj)
                    if j > 0:
                        emit_pv(b, blk, j - 1, psa, ees.pop(j - 1))
                    # spread fillers evenly through the j loop
                    while fi < len(fillers) * (j + 1) // nj:
                        fillers[fi]()
                        fi += 1
                emit_pv(b, blk, jlast, psa, ees.pop(jlast))
                while fi < len(fillers):
                    fillers[fi]()
                    fi += 1
                emit_norm(b, blk, psa)

            # ================= schedule =================
            # QKV(b0) k/q quarter 0 only -- the first scores (and so Act's
            # exp stream) start as early as possible; each block's v quarter
            # rides its `pre` hook between the first scores and the
            # transposes that consume it.
            for kind in ("k", "q"):
                emit_qkv_quarter(0, kind, 0)

            def q_fill(b, kind, qi):
                return lambda: emit_qkv_quarter(b, kind, qi)

            # Invariants (PE executes in emission order, so a block's vhat
            # transposes and first scores must FOLLOW the quarters they
            # read): q quarter i+1 emitted during block i; v quarter i via
            # the block's `pre` hook; k quarter i+1 may ride in block i+1's
            # own fillers (only needed from j=4i+4).
            fills = {
                0: [q_fill(0, "q", 1)],
                1: [q_fill(0, "k", 1), q_fill(0, "q", 2)],
                2: [q_fill(0, "k", 2), q_fill(0, "q", 3),
                    q_fill(1, "v", 0), q_fill(1, "v", 1)],
                3: [q_fill(0, "k", 3), q_fill(1, "v", 2), q_fill(1, "v", 3)]
                   + [q_fill(1, "k", qi) for qi in range(4)]
                   + [q_fill(1, "q", qi) for qi in range(4)],
            }
            pres = {blk: [q_fill(0, "v", blk)] for blk in range(4)}
            for blk in range(4):
                emit_attn_block(0, blk, fills[blk], pre=pres[blk])

            # attn(b1) descending (largest block first -> smallest block and
            # its proj become the epilogue), proj(b0) as PE filler.
            proj0 = [(blk, dt8) for blk in range(4) for dt8 in range(8)]
            fill_per_blk = {3: 11, 2: 9, 1: 7, 0: 5}
            pi = 0
            for blk in (3, 2, 1, 0):
                fillers = []
                for _ in range(fill_per_blk[blk]):
                    b0blk, dt8 = proj0[pi]
                    pi += 1
                    fillers.append(
                        lambda bb=b0blk, d=dt8: emit_proj_tile(0, bb, d))
                emit_attn_block(1, blk, fillers)
                # proj(b1) for this block right after its norm
                for dt8 in range(8):
                    emit_proj_tile(1, blk, dt8)
    nc.compile()
    return nc


def _get_nc():
    if "nc" not in _CACHE:
        _CACHE["nc"] = build_nc()
    return _CACHE["nc"]


def _bf16(a):
    import ml_dtypes
    return np.ascontiguousarray(a.astype(ml_dtypes.bfloat16))


def prep_w(w):
    # [1024, 128] -> [128(p), 8(d), 128(m)] so the SBUF load is contiguous
    return _bf16(np.ascontiguousarray(w.reshape(8, 128, 128).transpose(1, 0, 2)))


def make_in_maps(x, W_attn, b_attn, W_proj):
    x = np.ascontiguousarray(x, dtype=np.float32)
    xT = _bf16(np.ascontiguousarray(x.transpose(0, 2, 1)))

    p = np.arange(128)
    negm2 = np.where(p[:, None] <= p[None, :], 1.0, 0.0).astype(np.float32)
    ident = np.eye(128, dtype=np.float32)
    ones = np.ones((128, 32), np.float32)
    l2r = np.zeros((2, 128), np.float32)
    l2r[0, :] = 1.0

    in_maps = []
    for c in range(NCORE):
        col0 = HD * HPC * c
        in_maps.append({
            "xT": xT,
            "wq": prep_w(W_attn[:, col0:col0 + 128]),
            "wk": prep_w(W_attn[:, D + col0:D + col0 + 128]),
            "wv": prep_w(W_attn[:, 2 * D + col0:2 * D + col0 + 128]),
            "bq": np.ascontiguousarray(b_attn[col0:col0 + 128].reshape(128, 1)).astype(np.float32),
            "bk": np.ascontiguousarray(b_attn[D + col0:D + col0 + 128].reshape(128, 1)).astype(np.float32),
            "bv": np.ascontiguousarray(b_attn[2 * D + col0:2 * D + col0 + 128].reshape(128, 1)).astype(np.float32),
            "wp": _bf16(np.ascontiguousarray(W_proj[128 * c:128 * (c + 1), :])),
            "negm2": _bf16(negm2),
            "ident": _bf16(ident),
            "ones": _bf16(ones),
            "l2r": np.ascontiguousarray(l2r),
        })
    return in_maps


def gather(results, b_proj):
    acc = np.zeros((B, D, S), np.float32)
    for r in results:
        acc += np.asarray(r["yT"], np.float32)
    out = acc.transpose(0, 2, 1) + np.asarray(b_proj, np.float32)[None, None, :]
    return np.ascontiguousarray(out.astype(np.float32))


def kernel(x, W_attn, b_attn, W_proj, b_proj, _trace=False, _trace_kwargs=None):
    nc = _get_nc()
    in_maps = make_in_maps(np.asarray(x), np.asarray(W_attn),
                           np.asarray(b_attn), np.asarray(W_proj))
    res = run_bass_kernel_spmd(
        nc, in_maps, list(range(NCORE)), trace=_trace, **(_trace_kwargs or {})
    )
    out = gather(res.results, np.asarray(b_proj))
    if _trace:
        kernel.last_result = res
    return out


# revision 37
# speedup vs baseline: 1.1861x; 1.1861x over previous
"""Trainium2 Bass kernel for nn_Attention (B=2, S=2048, D=1024, H=16, causal).

Sharding: head-parallel across 8 NeuronCores - 2 heads per core. Each core:
  1. computes qT/kT/vT for its 2 heads from the full xT (QKV projection,
     transposed layout [128 = 2*hd, S]), in bf16,
  2. runs causal attention per head with scores in transposed orientation
     (sT[sj, si]) so the PV matmul needs no P transpose; the softmax
     denominator comes free as an extra ones-column in the V operand,
  3. multiplies by its 128-row slice of W_proj producing a partial output
     yT_c [B, D, S] in bf16.
Host sums the 8 partials, adds b_proj, and transposes back to [B, S, D].

Scheduling: QKV(b1) matmul groups are interleaved into attention(b0) and
proj(b0) into attention(b1) so the PE never idles while the Activation
engine churns through the softmax exps.
"""
import sys

sys.path.insert(0, "/opt/trn_rl_repo")

import numpy as np
import concourse.bacc as bacc
import concourse.mybir as mybir
import concourse.tile as tile
from concourse.bass_utils import run_bass_kernel_spmd

dt = mybir.dt
BF16 = dt.bfloat16
F32R = dt.float32r
AF = mybir.ActivationFunctionType

B, S, D, H = 2, 2048, 1024, 16
HD = D // H            # 64
NCORE = 8
HPC = H // NCORE       # 2 heads per core

_CACHE = {}


def build_nc():
    nc = bacc.Bacc("TRN2", target_bir_lowering=False, debug=False)

    xT_d = nc.dram_tensor("xT", [B, D, S], BF16, kind="ExternalInput")
    wq_d = nc.dram_tensor("wq", [128, 8, 128], BF16, kind="ExternalInput")
    wk_d = nc.dram_tensor("wk", [128, 8, 128], BF16, kind="ExternalInput")
    wv_d = nc.dram_tensor("wv", [128, 8, 128], BF16, kind="ExternalInput")
    bq_d = nc.dram_tensor("bq", [128, 1], dt.float32, kind="ExternalInput")
    bk_d = nc.dram_tensor("bk", [128, 1], dt.float32, kind="ExternalInput")
    bv_d = nc.dram_tensor("bv", [128, 1], dt.float32, kind="ExternalInput")
    wp_d = nc.dram_tensor("wp", [128, D], BF16, kind="ExternalInput")
    negm_d = nc.dram_tensor("negm2", [128, 128], BF16, kind="ExternalInput")
    id_d = nc.dram_tensor("ident", [128, 128], BF16, kind="ExternalInput")
    ones_d = nc.dram_tensor("ones", [128, 32], BF16, kind="ExternalInput")
    l2r_d = nc.dram_tensor("l2r", [2, 128], F32R, kind="ExternalInput")
    yT_d = nc.dram_tensor("yT", [B, D, S], BF16, kind="ExternalOutput")

    with tile.TileContext(nc) as tc:
        with (
            tc.tile_pool(name="consts", bufs=1) as consts,
            tc.tile_pool(name="xpool", bufs=2) as xpool,
            tc.tile_pool(name="qkv", bufs=2) as qkvp,
            tc.tile_pool(name="epool", bufs=3) as epool,
            tc.tile_pool(name="ypool", bufs=6) as ypool,
            tc.tile_pool(name="rpool", bufs=3) as rpool,
            tc.tile_pool(name="ps_mm2", bufs=3, space="PSUM") as ps_mm2,
            tc.tile_pool(name="ps_acc", bufs=2, space="PSUM") as ps_acc,
        ):
            # ---- constants / weights: issued on the scalar (Act) queue,
            # which is otherwise idle until the first exps ~15us in. The
            # v-path weights go first (first QKV quarter is v).
            wqr = consts.tile([128, 8, 128], BF16, tag="wq")
            wkr = consts.tile([128, 8, 128], BF16, tag="wk")
            wvr = consts.tile([128, 8, 128], BF16, tag="wv")
            wpr = consts.tile([128, D], BF16, tag="wp")
            bq_sb = consts.tile([128, 1], dt.float32, tag="bq")
            bk_sb = consts.tile([128, 1], dt.float32, tag="bk")
            bv_sb = consts.tile([128, 1], dt.float32, tag="bv")
            negm2 = consts.tile([128, 128], BF16, tag="negm2")
            ident = consts.tile([128, 128], BF16, tag="ident")
            l2r = consts.tile([2, 128], F32R, tag="l2r")
            nc.scalar.dma_start(wvr[:], wv_d.ap()[:])
            nc.scalar.dma_start(bv_sb[:], bv_d.ap()[:])

            # preload the ln+exp activation table once: the auto-placement
            # pass picks per-func first-match tables and would thrash
            # Exp <-> Ln every block otherwise.
            from concourse.hw_specs import get_activation_tables
            _tabs = list(get_activation_tables(nc.m.arch).items())
            _combined = next(i for i, (_n, _s) in enumerate(_tabs)
                             if AF.Exp in _s and AF.Ln in _s)
            nc.scalar.add_instruction(mybir.InstLoadActFuncSet(
                name=nc.get_next_instruction_name(),
                act_func_set_id=_combined, ins=[], outs=[]))

            # ---- x tiles: [128, S] per (b, dtile); two half-loads each.
            # b0's first halves ride the fast-starting vector/scalar queues
            # (the sync queue only comes up ~7us in) so the first QKV
            # matmuls can begin ~2us after launch.
            xd = {}
            for b in range(B):
                for d8 in range(8):
                    t = xpool.tile([128, S], BF16, tag=f"x{d8}",
                                   name=f"x_{b}_{d8}")
                    for h in range(2):
                        eng = nc.sync if (d8 + h) % 2 == 0 else nc.gpsimd
                        eng.dma_start(
                            t[:, 1024 * h:1024 * (h + 1)],
                            xT_d.ap()[b, 128 * d8:128 * (d8 + 1),
                                      1024 * h:1024 * (h + 1)])
                    xd[(b, d8)] = t

            # remaining constants, after the critical x halves
            nc.scalar.dma_start(wkr[:], wk_d.ap()[:])
            nc.scalar.dma_start(bk_sb[:], bk_d.ap()[:])
            nc.scalar.dma_start(wqr[:], wq_d.ap()[:])
            nc.scalar.dma_start(bq_sb[:], bq_d.ap()[:])
            nc.scalar.dma_start(ident[:], id_d.ap()[:])
            nc.scalar.dma_start(negm2[:], negm_d.ap()[:])
            nc.scalar.dma_start(wpr[:], wp_d.ap()[:])
            nc.scalar.dma_start(l2r[:], l2r_d.ap()[:])

            qT, kT, vT, vhat, aT = {}, {}, {}, {}, {}
            for b in range(B):
                qT[b] = qkvp.tile([128, S], BF16, tag="qT", name=f"qT_{b}")
                kT[b] = qkvp.tile([128, S], BF16, tag="kT", name=f"kT_{b}")
                vT[b] = qkvp.tile([128, S], BF16, tag="vT", name=f"vT_{b}")
                vhat[b] = qkvp.tile([128, 16, 130], BF16, tag="vhat",
                                    name=f"vhat_{b}")
                aT[b] = qkvp.tile([128, S], BF16, tag="aT", name=f"aT_{b}")
                # ones columns of vhat (softmax denominator rows).
                # NB: strided scatter -- must stay OFF the sync HWDGE queue
                # (produces zero columns there -> inf); scalar/gpsimd work.
                nc.scalar.dma_start(vhat[b][:, :, 64], ones_d.ap()[:, 0:16])
                nc.scalar.dma_start(vhat[b][:, :, 129], ones_d.ap()[:, 16:32])

            KINDS = {"q": (wqr, bq_sb, qT), "k": (wkr, bk_sb, kT),
                     "v": (wvr, bv_sb, vT)}

            def emit_qkv_quarter(b, kind, qi):
                """One [128, 512] quarter of q/k/v for batch b; 8 matmuls +
                one DVE bias-add drain."""
                w_r, bias, dst = KINDS[kind]
                pp = ps_mm2.tile([128, 2, 512], dt.float32, tag="mm2",
                                 name=f"qkv_{b}_{kind}_{qi}")
                cols = slice(512 * qi, 512 * (qi + 1))
                for d8 in range(8):
                    nc.tensor.matmul(
                        pp[:, 0, :],
                        w_r[:, d8, :],
                        xd[(b, d8)][:, cols],
                        start=(d8 == 0),
                        stop=(d8 == 7),
                    )
                with nc.allow_low_precision(reason="bf16 qkv bias"):
                    nc.vector.tensor_scalar_add(
                        dst[b][:, cols], pp[:, 0, :], bias[:, 0:1])

            def emit_vhat(b, j):
                """v natural-orientation tile j + ones column via PE
                transpose."""
                pst = ps_mm2.tile([128, 128], BF16, tag="mm2",
                                  name=f"tr_{b}_{j}")
                nc.tensor.transpose(
                    pst[:], vT[b][:, 128 * j:128 * (j + 1)], ident[:]
                )
                with nc.allow_low_precision(reason="bf16 vhat"):
                    nc.vector.tensor_copy(vhat[b][:, j, 0:64], pst[:, 0:64])
                    nc.vector.tensor_copy(vhat[b][:, j, 65:129],
                                          pst[:, 64:128])

            def emit_scores(b, blk, j):
                """Scores sT[sj, si] for both heads of j-tile vs query block
                blk, exp'd into a bf16 ee tile (diag tiles causally
                masked)."""
                si0 = 512 * blk
                off = max(0, 128 * (j - 4 * blk))
                w = 512 - off
                pp = ps_mm2.tile([128, 2, 512], dt.float32, tag="mm2",
                                 name=f"pp_{b}_{blk}_{j}")
                for hl in range(HPC):
                    rows = slice(64 * hl, 64 * (hl + 1))
                    nc.tensor.matmul(
                        pp[:, hl, 0:w],
                        kT[b][rows, 128 * j:128 * (j + 1)],
                        qT[b][rows, si0 + off:si0 + 512],
                        start=True,
                        stop=True,
                    )
                ee = epool.tile([128, 2, 512], BF16, tag="eT",
                                name=f"ee_{b}_{blk}_{j}")
                nc.scalar.activation(
                    ee[:, :, 0:w], pp[:, :, 0:w], AF.Exp, scale=0.125
                )
                if j >= 4 * blk:
                    with nc.allow_low_precision(reason="causal mask"):
                        for hl in range(HPC):
                            nc.vector.tensor_mul(
                                ee[:, hl, 0:128], ee[:, hl, 0:128], negm2[:]
                            )
                return ee

            def emit_pv(b, blk, j, psa, ee):
                off = max(0, 128 * (j - 4 * blk))
                w = 512 - off
                jlast = 4 * blk + 3
                for hl in range(HPC):
                    nc.tensor.matmul(
                        psa[hl][:, off:512],
                        vhat[b][:, j, 65 * hl:65 * hl + 65],
                        ee[:, hl, 0:w],
                        start=(j == 0),
                        stop=(j == jlast),
                    )

            def emit_norm(b, blk, psa):
                """aT[:, blk] = psa rows / softmax denominator (psa row 64),
                via DVE reciprocal + PE broadcast."""
                si0 = 512 * blk
                # 1/d via exp(-ln d): Ln rows, col-tiled PE broadcast (the
                # two matmuls run concurrently), one Exp drain -- all on the
                # single preloaded ln+exp act table.
                lnl = [
                    rpool.tile([1, 512], F32R, tag=f"lnl{hl}",
                               name=f"lnl_{b}_{blk}_{hl}")
                    for hl in range(HPC)
                ]
                for hl in range(HPC):
                    nc.scalar.activation(lnl[hl][:], psa[hl][64:65, :], AF.Ln)
                rec_sb = []
                for hl in range(HPC):
                    psb = ps_mm2.tile([64, 512], dt.float32, tag="mm2",
                                      name=f"psb_{b}_{blk}_{hl}")
                    nc.tensor.matmul(psb[:], l2r[0:1, 0:64], lnl[hl][:],
                                     start=True, stop=True)
                    rs = rpool.tile([64, 512], BF16, tag=f"recs{hl}",
                                    name=f"recs_{b}_{blk}_{hl}")
                    nc.scalar.activation(rs[:], psb[:], AF.Exp, scale=-1.0)
                    rec_sb.append(rs)
                with nc.allow_low_precision(reason="bf16 attn normalize"):
                    for hl in range(HPC):
                        p0 = 64 * hl
                        nc.vector.tensor_mul(
                            aT[b][p0:p0 + 64, si0:si0 + 512],
                            psa[hl][0:64, :],
                            rec_sb[hl][:],
                        )

            _proj_n = [0]

            def emit_proj_tile(b, blk, dtile):
                si0 = 512 * blk
                ps = ps_mm2.tile([128, 512], dt.float32, tag="mm2",
                                 name=f"psp_{b}_{blk}_{dtile}")
                nc.tensor.matmul(
                    ps[:],
                    wpr[:, 128 * dtile:128 * (dtile + 1)],
                    aT[b][:, si0:si0 + 512],
                    start=True,
                    stop=True,
                )
                y_sb = ypool.tile([128, 512], BF16, tag="y",
                                  name=f"y_{b}_{blk}_{dtile}")
                n = _proj_n[0]
                _proj_n[0] += 1
                # tail-most b1 block drains alternate Act/DVE (Act idle
                # once exps are done); GPSIMD has no PSUM port.
                with nc.allow_low_precision(reason="bf16 y drain"):
                    if b == 1 and blk == 0 and dtile % 2 == 0:
                        nc.scalar.activation(y_sb[:], ps[:], AF.Copy)
                    else:
                        nc.vector.tensor_copy(y_sb[:], ps[:])
                dma_eng = nc.sync if n % 2 == 0 else nc.gpsimd
                dma_eng.dma_start(
                    yT_d.ap()[b, 128 * dtile:128 * (dtile + 1),
                              si0:si0 + 512],
                    y_sb[:],
                )

            _vhat_done = {0: set(), 1: set()}

            def emit_attn_block(b, blk, fillers, pre=()):
                """One 512-query attention block; `fillers` is a list of
                zero-arg emit callbacks sprinkled into the PE stream to keep
                the PE busy while Act runs the exps. `pre` callbacks run
                right after the first scores, before the vhat transposes
                (e.g. the v quarter those transposes consume)."""
                jlast = 4 * blk + 3
                psa = [
                    ps_acc.tile([65, 512], dt.float32, tag="acc",
                                name=f"psa_{b}_{blk}_{hl}")
                    for hl in range(HPC)
                ]
                fi = 0
                nj = jlast + 1
                ees = {}
                for j in range(nj):
                    ees[j] = emit_scores(b, blk, j)
                    if j == 0:
                        for p in pre:
                            p()
                        # vhat transposes after the first scores so Act's
                        # first exp of the block isn't delayed by them
                        for jj in range(jlast + 1):
                            if jj not in _vhat_done[b]:
                                emit_vhat(b, jj)
                                _vhat_done[b].add(jj)
                    if j > 0:
                        emit_pv(b, blk, j - 1, psa, ees.pop(j - 1))
                    # spread fillers evenly through the j loop
                    while fi < len(fillers) * (j + 1) // nj:
                        fillers[fi]()
                        fi += 1
                emit_pv(b, blk, jlast, psa, ees.pop(jlast))
                while fi < len(fillers):
                    fillers[fi]()
                    fi += 1
                emit_norm(b, blk, psa)

            # ================= schedule =================
            # QKV(b0) quarter 0 (v first: vhat feeds PV).
            for kind in ("v", "k", "q"):
                emit_qkv_quarter(0, kind, 0)

            def q_fill(b, kind, qi):
                return lambda: emit_qkv_quarter(b, kind, qi)

            # Invariants (PE executes in emission order, so a block's vhat
            # transposes and first scores must FOLLOW the quarters they
            # read): q quarter i+1 emitted during block i; v quarter i via
            # the block's `pre` hook; k quarter i+1 may ride in block i+1's
            # own fillers (only needed from j=4i+4).
            fills = {
                0: [q_fill(0, "v", 1), q_fill(0, "q", 1)],
                1: [q_fill(0, "k", 1), q_fill(0, "v", 2), q_fill(0, "q", 2)],
                2: [q_fill(0, "k", 2), q_fill(0, "v", 3), q_fill(0, "q", 3),
                    q_fill(1, "v", 0), q_fill(1, "v", 1)],
                3: [q_fill(0, "k", 3), q_fill(1, "v", 2), q_fill(1, "v", 3)]
                   + [q_fill(1, "k", qi) for qi in range(4)]
                   + [q_fill(1, "q", qi) for qi in range(4)],
            }
            for blk in range(4):
                emit_attn_block(0, blk, fills[blk])

            # attn(b1) descending (largest block first -> smallest block and
            # its proj become the epilogue), proj(b0) as PE filler.
            proj0 = [(blk, dt8) for blk in range(4) for dt8 in range(8)]
            fill_per_blk = {3: 11, 2: 9, 1: 7, 0: 5}
            pi = 0
            for blk in (3, 2, 1, 0):
                fillers = []
                for _ in range(fill_per_blk[blk]):
                    b0blk, dt8 = proj0[pi]
                    pi += 1
                    fillers.append(
                        lambda bb=b0blk, d=dt8: emit_proj_tile(0, bb, d))
                emit_attn_block(1, blk, fillers)
                # proj(b1) for this block right after its norm
                for dt8 in range(8):
                    emit_proj_tile(1, blk, dt8)
    nc.compile()
    return nc


def _get_nc():
    if "nc" not in _CACHE:
        _CACHE["nc"] = build_nc()
    return _CACHE["nc"]


def _bf16(a):
    import ml_dtypes
    return np.ascontiguousarray(a.astype(ml_dtypes.bfloat16))


def prep_w(w):
    # [1024, 128] -> [128(p), 8(d), 128(m)] so the SBUF load is contiguous
    return _bf16(np.ascontiguousarray(w.reshape(8, 128, 128).transpose(1, 0, 2)))


def make_in_maps(x, W_attn, b_attn, W_proj):
    x = np.ascontiguousarray(x, dtype=np.float32)
    xT = _bf16(np.ascontiguousarray(x.transpose(0, 2, 1)))

    p = np.arange(128)
    negm2 = np.where(p[:, None] <= p[None, :], 1.0, 0.0).astype(np.float32)
    ident = np.eye(128, dtype=np.float32)
    ones = np.ones((128, 32), np.float32)
    l2r = np.zeros((2, 128), np.float32)
    l2r[0, :] = 1.0

    in_maps = []
    for c in range(NCORE):
        col0 = HD * HPC * c
        in_maps.append({
            "xT": xT,
            "wq": prep_w(W_attn[:, col0:col0 + 128]),
            "wk": prep_w(W_attn[:, D + col0:D + col0 + 128]),
            "wv": prep_w(W_attn[:, 2 * D + col0:2 * D + col0 + 128]),
            "bq": np.ascontiguousarray(b_attn[col0:col0 + 128].reshape(128, 1)).astype(np.float32),
            "bk": np.ascontiguousarray(b_attn[D + col0:D + col0 + 128].reshape(128, 1)).astype(np.float32),
            "bv": np.ascontiguousarray(b_attn[2 * D + col0:2 * D + col0 + 128].reshape(128, 1)).astype(np.float32),
            "wp": _bf16(np.ascontiguousarray(W_proj[128 * c:128 * (c + 1), :])),
            "negm2": _bf16(negm2),
            "ident": _bf16(ident),
            "ones": _bf16(ones),
            "l2r": np.ascontiguousarray(l2r),
        })
    return in_maps


def gather(results, b_proj):
    acc = np.zeros((B, D, S), np.float32)
    for r in results:
        acc += np.asarray(r["yT"], np.float32)
    out = acc.transpose(0, 2, 1) + np.asarray(b_proj, np.float32)[None, None, :]
    return np.ascontiguousarray(out.astype(np.float32))


def kernel(x, W_attn, b_attn, W_proj, b_proj, _trace=False, _trace_kwargs=None):
    nc = _get_nc()
    in_maps = make_in_maps(np.asarray(x), np.asarray(W_attn),
                           np.asarray(b_attn), np.asarray(W_proj))
    res = run_bass_kernel_spmd(
        nc, in_maps, list(range(NCORE)), trace=_trace, **(_trace_kwargs or {})
    )
    out = gather(res.results, np.asarray(b_proj))
    if _trace:
        kernel.last_result = res
    return out


# revision 39
# speedup vs baseline: 1.1940x; 1.0067x over previous
"""Trainium2 Bass kernel for nn_Attention (B=2, S=2048, D=1024, H=16, causal).

Sharding: head-parallel across 8 NeuronCores - 2 heads per core. Each core:
  1. computes qT/kT/vT for its 2 heads from the full xT (QKV projection,
     transposed layout [128 = 2*hd, S]), in bf16,
  2. runs causal attention per head with scores in transposed orientation
     (sT[sj, si]) so the PV matmul needs no P transpose; the softmax
     denominator comes free as an extra ones-column in the V operand,
  3. multiplies by its 128-row slice of W_proj producing a partial output
     yT_c [B, D, S] in bf16.
Host sums the 8 partials, adds b_proj, and transposes back to [B, S, D].

Scheduling: QKV(b1) matmul groups are interleaved into attention(b0) and
proj(b0) into attention(b1) so the PE never idles while the Activation
engine churns through the softmax exps.
"""
import sys

sys.path.insert(0, "/opt/trn_rl_repo")

import numpy as np
import concourse.bacc as bacc
import concourse.mybir as mybir
import concourse.tile as tile
from concourse.bass_utils import run_bass_kernel_spmd

dt = mybir.dt
BF16 = dt.bfloat16
F32R = dt.float32r
AF = mybir.ActivationFunctionType

B, S, D, H = 2, 2048, 1024, 16
HD = D // H            # 64
NCORE = 8
HPC = H // NCORE       # 2 heads per core

_CACHE = {}


def build_nc():
    nc = bacc.Bacc("TRN2", target_bir_lowering=False, debug=False)

    xT_d = nc.dram_tensor("xT", [B, D, S], BF16, kind="ExternalInput")
    wq_d = nc.dram_tensor("wq", [128, 8, 128], BF16, kind="ExternalInput")
    wk_d = nc.dram_tensor("wk", [128, 8, 128], BF16, kind="ExternalInput")
    wv_d = nc.dram_tensor("wv", [128, 8, 128], BF16, kind="ExternalInput")
    bq_d = nc.dram_tensor("bq", [128, 1], dt.float32, kind="ExternalInput")
    bk_d = nc.dram_tensor("bk", [128, 1], dt.float32, kind="ExternalInput")
    bv_d = nc.dram_tensor("bv", [128, 1], dt.float32, kind="ExternalInput")
    wp_d = nc.dram_tensor("wp", [128, D], BF16, kind="ExternalInput")
    negm_d = nc.dram_tensor("negm2", [128, 128], BF16, kind="ExternalInput")
    id_d = nc.dram_tensor("ident", [128, 128], BF16, kind="ExternalInput")
    ones_d = nc.dram_tensor("ones", [128, 32], BF16, kind="ExternalInput")
    l2r_d = nc.dram_tensor("l2r", [2, 128], F32R, kind="ExternalInput")
    yT_d = nc.dram_tensor("yT", [B, D, S], BF16, kind="ExternalOutput")

    with tile.TileContext(nc) as tc:
        with (
            tc.tile_pool(name="consts", bufs=1) as consts,
            tc.tile_pool(name="xpool", bufs=2) as xpool,
            tc.tile_pool(name="qkv", bufs=2) as qkvp,
            tc.tile_pool(name="epool", bufs=3) as epool,
            tc.tile_pool(name="ypool", bufs=6) as ypool,
            tc.tile_pool(name="rpool", bufs=3) as rpool,
            tc.tile_pool(name="ps_mm2", bufs=3, space="PSUM") as ps_mm2,
            tc.tile_pool(name="ps_acc", bufs=2, space="PSUM") as ps_acc,
        ):
            # ---- constants / weights: issued on the scalar (Act) queue,
            # which is otherwise idle until the first exps ~15us in. The
            # v-path weights go first (first QKV quarter is v).
            wqr = consts.tile([128, 8, 128], BF16, tag="wq")
            wkr = consts.tile([128, 8, 128], BF16, tag="wk")
            wvr = consts.tile([128, 8, 128], BF16, tag="wv")
            wpr = consts.tile([128, D], BF16, tag="wp")
            bq_sb = consts.tile([128, 1], dt.float32, tag="bq")
            bk_sb = consts.tile([128, 1], dt.float32, tag="bk")
            bv_sb = consts.tile([128, 1], dt.float32, tag="bv")
            negm2 = consts.tile([128, 128], BF16, tag="negm2")
            ident = consts.tile([128, 128], BF16, tag="ident")
            l2r = consts.tile([2, 128], F32R, tag="l2r")
            nc.scalar.dma_start(wvr[:], wv_d.ap()[:])
            nc.scalar.dma_start(bv_sb[:], bv_d.ap()[:])

            # preload the ln+exp activation table once: the auto-placement
            # pass picks per-func first-match tables and would thrash
            # Exp <-> Ln every block otherwise.
            from concourse.hw_specs import get_activation_tables
            _tabs = list(get_activation_tables(nc.m.arch).items())
            _combined = next(i for i, (_n, _s) in enumerate(_tabs)
                             if AF.Exp in _s and AF.Ln in _s)
            nc.scalar.add_instruction(mybir.InstLoadActFuncSet(
                name=nc.get_next_instruction_name(),
                act_func_set_id=_combined, ins=[], outs=[]))

            # ---- x tiles: [128, S] per (b, dtile); two half-loads each.
            # b0's first halves ride the fast-starting vector/scalar queues
            # (the sync queue only comes up ~7us in) so the first QKV
            # matmuls can begin ~2us after launch.
            xd = {}
            for b in range(B):
                for d8 in range(8):
                    t = xpool.tile([128, S], BF16, tag=f"x{d8}",
                                   name=f"x_{b}_{d8}")
                    for h in range(2):
                        eng = nc.sync if (d8 + h) % 2 == 0 else nc.gpsimd
                        eng.dma_start(
                            t[:, 1024 * h:1024 * (h + 1)],
                            xT_d.ap()[b, 128 * d8:128 * (d8 + 1),
                                      1024 * h:1024 * (h + 1)])
                    xd[(b, d8)] = t

            # remaining constants, after the critical x halves
            nc.scalar.dma_start(wkr[:], wk_d.ap()[:])
            nc.scalar.dma_start(bk_sb[:], bk_d.ap()[:])
            nc.scalar.dma_start(wqr[:], wq_d.ap()[:])
            nc.scalar.dma_start(bq_sb[:], bq_d.ap()[:])
            nc.scalar.dma_start(ident[:], id_d.ap()[:])
            nc.scalar.dma_start(negm2[:], negm_d.ap()[:])
            nc.scalar.dma_start(wpr[:], wp_d.ap()[:])
            nc.scalar.dma_start(l2r[:], l2r_d.ap()[:])

            qT, kT, vT, vhat, aT = {}, {}, {}, {}, {}
            for b in range(B):
                qT[b] = qkvp.tile([128, S], BF16, tag="qT", name=f"qT_{b}")
                kT[b] = qkvp.tile([128, S], BF16, tag="kT", name=f"kT_{b}")
                vT[b] = qkvp.tile([128, S], BF16, tag="vT", name=f"vT_{b}")
                vhat[b] = qkvp.tile([128, 16, 130], BF16, tag="vhat",
                                    name=f"vhat_{b}")
                aT[b] = qkvp.tile([128, S], BF16, tag="aT", name=f"aT_{b}")
                # ones columns of vhat (softmax denominator rows).
                # NB: strided scatter -- must stay OFF the sync HWDGE queue
                # (produces zero columns there -> inf); scalar/gpsimd work.
                nc.scalar.dma_start(vhat[b][:, :, 64], ones_d.ap()[:, 0:16])
                nc.scalar.dma_start(vhat[b][:, :, 129], ones_d.ap()[:, 16:32])

            KINDS = {"q": (wqr, bq_sb, qT), "k": (wkr, bk_sb, kT),
                     "v": (wvr, bv_sb, vT)}

            def emit_qkv_quarter(b, kind, qi):
                """One [128, 512] quarter of q/k/v for batch b; 8 matmuls +
                one DVE bias-add drain."""
                w_r, bias, dst = KINDS[kind]
                pp = ps_mm2.tile([128, 2, 512], dt.float32, tag="mm2",
                                 name=f"qkv_{b}_{kind}_{qi}")
                cols = slice(512 * qi, 512 * (qi + 1))
                for d8 in range(8):
                    nc.tensor.matmul(
                        pp[:, 0, :],
                        w_r[:, d8, :],
                        xd[(b, d8)][:, cols],
                        start=(d8 == 0),
                        stop=(d8 == 7),
                    )
                with nc.allow_low_precision(reason="bf16 qkv bias"):
                    nc.vector.tensor_scalar_add(
                        dst[b][:, cols], pp[:, 0, :], bias[:, 0:1])

            def emit_vhat(b, j):
                """v natural-orientation tile j + ones column via PE
                transpose."""
                pst = ps_mm2.tile([128, 128], BF16, tag="mm2",
                                  name=f"tr_{b}_{j}")
                nc.tensor.transpose(
                    pst[:], vT[b][:, 128 * j:128 * (j + 1)], ident[:]
                )
                with nc.allow_low_precision(reason="bf16 vhat"):
                    nc.vector.tensor_copy(vhat[b][:, j, 0:64], pst[:, 0:64])
                    nc.vector.tensor_copy(vhat[b][:, j, 65:129],
                                          pst[:, 64:128])

            def emit_scores(b, blk, j):
                """Scores sT[sj, si] for both heads of j-tile vs query block
                blk, exp'd into a bf16 ee tile (diag tiles causally
                masked)."""
                si0 = 512 * blk
                off = max(0, 128 * (j - 4 * blk))
                w = 512 - off
                pp = ps_mm2.tile([128, 2, 512], dt.float32, tag="mm2",
                                 name=f"pp_{b}_{blk}_{j}")
                for hl in range(HPC):
                    rows = slice(64 * hl, 64 * (hl + 1))
                    nc.tensor.matmul(
                        pp[:, hl, 0:w],
                        kT[b][rows, 128 * j:128 * (j + 1)],
                        qT[b][rows, si0 + off:si0 + 512],
                        start=True,
                        stop=True,
                    )
                ee = epool.tile([128, 2, 512], BF16, tag="eT",
                                name=f"ee_{b}_{blk}_{j}")
                nc.scalar.activation(
                    ee[:, :, 0:w], pp[:, :, 0:w], AF.Exp, scale=0.125
                )
                if j >= 4 * blk:
                    with nc.allow_low_precision(reason="causal mask"):
                        for hl in range(HPC):
                            nc.vector.tensor_mul(
                                ee[:, hl, 0:128], ee[:, hl, 0:128], negm2[:]
                            )
                return ee

            def emit_pv(b, blk, j, psa, ee):
                off = max(0, 128 * (j - 4 * blk))
                w = 512 - off
                jlast = 4 * blk + 3
                for hl in range(HPC):
                    nc.tensor.matmul(
                        psa[hl][:, off:512],
                        vhat[b][:, j, 65 * hl:65 * hl + 65],
                        ee[:, hl, 0:w],
                        start=(j == 0),
                        stop=(j == jlast),
                    )

            def emit_norm(b, blk, psa):
                """aT[:, blk] = psa rows / softmax denominator (psa row 64),
                via DVE reciprocal + PE broadcast."""
                si0 = 512 * blk
                # 1/d via exp(-ln d): Ln rows, col-tiled PE broadcast (the
                # two matmuls run concurrently), one Exp drain -- all on the
                # single preloaded ln+exp act table.
                lnl = [
                    rpool.tile([1, 512], F32R, tag=f"lnl{hl}",
                               name=f"lnl_{b}_{blk}_{hl}")
                    for hl in range(HPC)
                ]
                for hl in range(HPC):
                    nc.scalar.activation(lnl[hl][:], psa[hl][64:65, :], AF.Ln)
                rec_sb = []
                for hl in range(HPC):
                    psb = ps_mm2.tile([64, 512], dt.float32, tag="mm2",
                                      name=f"psb_{b}_{blk}_{hl}")
                    nc.tensor.matmul(psb[:], l2r[0:1, 0:64], lnl[hl][:],
                                     start=True, stop=True)
                    rs = rpool.tile([64, 512], BF16, tag=f"recs{hl}",
                                    name=f"recs_{b}_{blk}_{hl}")
                    nc.scalar.activation(rs[:], psb[:], AF.Exp, scale=-1.0)
                    rec_sb.append(rs)
                with nc.allow_low_precision(reason="bf16 attn normalize"):
                    for hl in range(HPC):
                        p0 = 64 * hl
                        nc.vector.tensor_mul(
                            aT[b][p0:p0 + 64, si0:si0 + 512],
                            psa[hl][0:64, :],
                            rec_sb[hl][:],
                        )

            _proj_n = [0]

            def emit_proj_tile(b, blk, dtile):
                si0 = 512 * blk
                ps = ps_mm2.tile([128, 512], dt.float32, tag="mm2",
                                 name=f"psp_{b}_{blk}_{dtile}")
                nc.tensor.matmul(
                    ps[:],
                    wpr[:, 128 * dtile:128 * (dtile + 1)],
                    aT[b][:, si0:si0 + 512],
                    start=True,
                    stop=True,
                )
                y_sb = ypool.tile([128, 512], BF16, tag="y",
                                  name=f"y_{b}_{blk}_{dtile}")
                n = _proj_n[0]
                _proj_n[0] += 1
                # tail-most b1 block drains alternate Act/DVE (Act idle
                # once exps are done); GPSIMD has no PSUM port.
                with nc.allow_low_precision(reason="bf16 y drain"):
                    if b == 1 and blk == 0 and dtile % 2 == 0:
                        nc.scalar.activation(y_sb[:], ps[:], AF.Copy)
                    else:
                        nc.vector.tensor_copy(y_sb[:], ps[:])
                dma_eng = nc.sync if n % 2 == 0 else nc.gpsimd
                dma_eng.dma_start(
                    yT_d.ap()[b, 128 * dtile:128 * (dtile + 1),
                              si0:si0 + 512],
                    y_sb[:],
                )

            _vhat_done = {0: set(), 1: set()}

            def emit_attn_block(b, blk, fillers, pre=()):
                """One 512-query attention block; `fillers` is a list of
                zero-arg emit callbacks sprinkled into the PE stream to keep
                the PE busy while Act runs the exps. `pre` callbacks run
                right after the first scores, before the vhat transposes
                (e.g. the v quarter those transposes consume)."""
                jlast = 4 * blk + 3
                psa = [
                    ps_acc.tile([65, 512], dt.float32, tag="acc",
                                name=f"psa_{b}_{blk}_{hl}")
                    for hl in range(HPC)
                ]
                fi = 0
                nj = jlast + 1
                ees = {}
                for j in range(nj):
                    ees[j] = emit_scores(b, blk, j)
                    if j == 0:
                        for p in pre:
                            p()
                        # vhat transposes after the first scores so Act's
                        # first exp of the block isn't delayed by them
                        for jj in range(jlast + 1):
                            if jj not in _vhat_done[b]:
                                emit_vhat(b, jj)
                                _vhat_done[b].add(jj)
                    if j > 0:
                        emit_pv(b, blk, j - 1, psa, ees.pop(j - 1))
                    # spread fillers evenly through the j loop
                    while fi < len(fillers) * (j + 1) // nj:
                        fillers[fi]()
                        fi += 1
                emit_pv(b, blk, jlast, psa, ees.pop(jlast))
                while fi < len(fillers):
                    fillers[fi]()
                    fi += 1
                emit_norm(b, blk, psa)

            # ================= schedule =================
            # QKV(b0) quarter 0 (v first: vhat feeds PV).
            for kind in ("v", "k", "q"):
                emit_qkv_quarter(0, kind, 0)

            def q_fill(b, kind, qi):
                return lambda: emit_qkv_quarter(b, kind, qi)

            # Invariants (PE executes in emission order, so a block's vhat
            # transposes and first scores must FOLLOW the quarters they
            # read): q quarter i+1 emitted during block i; v quarter i via
            # the block's `pre` hook; k quarter i+1 may ride in block i+1's
            # own fillers (only needed from j=4i+4).
            fills = {
                0: [q_fill(0, "v", 1), q_fill(0, "q", 1)],
                1: [q_fill(0, "k", 1), q_fill(0, "v", 2), q_fill(0, "q", 2)],
                2: [q_fill(0, "k", 2), q_fill(0, "v", 3), q_fill(0, "q", 3),
                    q_fill(1, "v", 0), q_fill(1, "v", 1)],
                3: [q_fill(0, "k", 3), q_fill(1, "v", 2), q_fill(1, "v", 3)]
                   + [q_fill(1, "k", qi) for qi in range(4)]
                   + [q_fill(1, "q", qi) for qi in range(4)],
            }
            for blk in range(4):
                emit_attn_block(0, blk, fills[blk])

            # attn(b1) descending (largest block first -> smallest block and
            # its proj become the epilogue), proj(b0) as PE filler.
            proj0 = [(blk, dt8) for blk in range(4) for dt8 in range(8)]
            fill_per_blk = {3: 11, 2: 9, 1: 7, 0: 5}
            pi = 0
            for blk in (3, 2, 1, 0):
                fillers = []
                for _ in range(fill_per_blk[blk]):
                    b0blk, dt8 = proj0[pi]
                    pi += 1
                    fillers.append(
                        lambda bb=b0blk, d=dt8: emit_proj_tile(0, bb, d))
                emit_attn_block(1, blk, fillers)
                # proj(b1) for this block right after its norm
                for dt8 in range(8):
                    emit_proj_tile(1, blk, dt8)
    nc.compile()
    return nc


def _get_nc():
    if "nc" not in _CACHE:
        _CACHE["nc"] = build_nc()
    return _CACHE["nc"]


def _bf16(a):
    import ml_dtypes
    return np.ascontiguousarray(a.astype(ml_dtypes.bfloat16))


def prep_w(w):
    # [1024, 128] -> [128(p), 8(d), 128(m)] so the SBUF load is contiguous
    return _bf16(np.ascontiguousarray(w.reshape(8, 128, 128).transpose(1, 0, 2)))


def make_in_maps(x, W_attn, b_attn, W_proj):
    x = np.ascontiguousarray(x, dtype=np.float32)
    xT = _bf16(np.ascontiguousarray(x.transpose(0, 2, 1)))

    p = np.arange(128)
    negm2 = np.where(p[:, None] <= p[None, :], 1.0, 0.0).astype(np.float32)
    ident = np.eye(128, dtype=np.float32)
    ones = np.ones((128, 32), np.float32)
    l2r = np.zeros((2, 128), np.float32)
    l2r[0, :] = 1.0

    in_maps = []
    for c in range(NCORE):
        col0 = HD * HPC * c
        in_maps.append({
            "xT": xT,
            "wq": prep_w(W_attn[:, col0:col0 + 128]),
            "wk": prep_w(W_attn[:, D + col0:D + col0 + 128]),
            "wv": prep_w(W_attn[:, 2 * D + col0:2 * D + col0 + 128]),
            "bq": np.ascontiguousarray(b_attn[col0:col0 + 128].reshape(128, 1)).astype(np.float32),
            "bk": np.ascontiguousarray(b_attn[D + col0:D + col0 + 128].reshape(128, 1)).astype(np.float32),
            "bv": np.ascontiguousarray(b_attn[2 * D + col0:2 * D + col0 + 128].reshape(128, 1)).astype(np.float32),
            "wp": _bf16(np.ascontiguousarray(W_proj[128 * c:128 * (c + 1), :])),
            "negm2": _bf16(negm2),
            "ident": _bf16(ident),
            "ones": _bf16(ones),
            "l2r": np.ascontiguousarray(l2r),
        })
    return in_maps


def gather(results, b_proj):
    acc = np.zeros((B, D, S), np.float32)
    for r in results:
        acc += np.asarray(r["yT"], np.float32)
    out = acc.transpose(0, 2, 1) + np.asarray(b_proj, np.float32)[None, None, :]
    return np.ascontiguousarray(out.astype(np.float32))


def kernel(x, W_attn, b_attn, W_proj, b_proj, _trace=False, _trace_kwargs=None):
    nc = _get_nc()
    in_maps = make_in_maps(np.asarray(x), np.asarray(W_attn),
                           np.asarray(b_attn), np.asarray(W_proj))
    res = run_bass_kernel_spmd(
        nc, in_maps, list(range(NCORE)), trace=_trace, **(_trace_kwargs or {})
    )
    out = gather(res.results, np.asarray(b_proj))
    if _trace:
        kernel.last_result = res
    return out
